# revision 31
# baseline (speedup 1.0000x reference)
"""CFAR OS-CA 2D detector kernel for Trainium2 (8 NeuronCores, Bass/Tile).

Algorithm
---------
reference: per (batch, vel) row of 1024 range cells (circular):
  OS stage: miu[r] = 8th largest of 32 training cells at r +- [5..20];
            os = alpha * miu
  CA stage: out[v] = mean over vel offsets +-[3..10] (circular) of os

Kernel strategy (per core = 2 batches = 512 rows, range on the free axis,
bf16 selection pipeline, ~1.6e-3 max rel err vs the fp32 reference):
  * van Herk / Gil-Werman on 16-blocks: for every block, the sorted top-8 of
    every prefix (and, scanning backward, every suffix) is built with a chain
    of 8 tensor_tensor_scan recurrences
        m_k[c] = min(max(x[c], state), m_{k-1}[c -/+ 1])
    which is provably the k-th-largest prefix recurrence; a single -1e30 pad
    column per block (17-column pages, re-cleaned by tiny gpsimd stripe
    memsets after each chain) both resets the state at block boundaries and
    serves as the "empty prefix" table entry.
  * every 16-wide window = one block suffix + next-block prefix; the top-8
    multiset of two sorted-desc 8-lists A,B is {max(A_i, B_{7-i})}
    ("valley"); a 3-stage bitonic merge sorts it descending -> W8(s) table.
    Valley/resort/final are emitted as multi-plane batched TT ops (bf16 ->
    2x DVE mode).
  * OS output: 8th largest of union of the two 16-windows at r-20 and r+5 =
    min_i max(W8(r-20)_i, W8(r+5)_{7-i}).
  * CA stage: circulant matmul on the tensor engine ([vel,vel] banded 0/1
    bf16 weights, fp32 PSUM accumulate; alpha/16 applied in fp32 on the
    ScalarE PSUM eviction) -- no transpose needed since vel sits on
    partitions.
Batch is pure data parallel across the 8 cores (no halo exchange needed).
Engines: DVE does all selection math (~190us busy); ScalarE casts/evicts,
GpSimd does pad-stripe memsets, PE does the CA matmul, all overlapped.
"""

import sys

if "/opt/trn_rl_repo" not in sys.path:
    sys.path.insert(0, "/opt/trn_rl_repo")

import math
from contextlib import ExitStack

import numpy as np

import concourse.mybir as mybir
from concourse import bacc, bass_utils
from concourse.ap import AP
from concourse.tile import TileContext

F32 = mybir.dt.float32
MIN = mybir.AluOpType.min
MAX = mybir.AluOpType.max
NEG = -1.0e30

# ---- module hyperparameters (match the nn.Module) ----
G = (2, 4)
T = (8, 16)
PFA = 1e-05
K_ORDER = 24
OS_N = 2 * T[1]          # 32
HR = G[1] + T[1]         # 20
HV = G[0] + T[0]         # 10


def _os_cfar_threshold(k, n, pfa):
    def log_factorial(n):
        n = n + 1
        if n < 9:
            return np.log(math.factorial(n))
        return 1 / 2 * (np.log(2 * np.pi) - np.log(n)) + n * (
            np.log(n + 1 / (12 * n - 1 / 10 / n)) - 1
        )

    def fun(k, n, t_os, pfa):
        return (
            log_factorial(n)
            - log_factorial(n - k)
            - np.sum(np.log(np.arange(n, n - k, -1) + t_os))
            - np.log(pfa)
        )

    t_max, t_min = 1e32, 1.0
    for _ in range(10000):
        m_n = t_max - fun(k, n, t_max, pfa) * (t_min - t_max) / (
            fun(k, n, t_min, pfa) - fun(k, n, t_max, pfa)
        )
        f_m_n = fun(k, n, m_n, pfa)
        if f_m_n == 0 or np.abs(t_max - t_min) < 0.0001:
            return m_n
        if fun(k, n, t_max, pfa) * f_m_n < 0:
            t_min = m_n
        elif fun(k, n, t_min, pfa) * f_m_n < 0:
            t_max = m_n
        else:
            break
    raise ValueError("CFAR threshold did not converge.")


OS_ALPHA = float(np.sqrt(_os_cfar_threshold(K_ORDER, OS_N, PFA)))

# ---- problem/shard geometry ----
B, V, R = 16, 256, 1024
NCORES = 8
BPC = B // NCORES        # batches per core
ROWS = BPC * V           # 512 rows per core
NT = ROWS // 128         # 4 partition tiles
HALO = 32
XC = R + 2 * HALO        # 1088 haloed columns
NBLK = XC // 16          # 68 16-blocks
# page layout: [p0, y0..y15] per block; a single pad column per block is
# kept clean (-1e30) by a tiny gpsimd stripe-memset after each scan chain,
# which both resets the next chain at block starts and serves as the
# empty-prefix entry for the window merge.
PADS = 1
PGW = PADS + 16          # 17
W1H = NBLK * PGW         # 1156
VB = 67                  # window-table blocks
VW = VB * 16             # 1072 window-start columns (s = col - 32)
BF16 = mybir.dt.bfloat16


def _ca_weights() -> np.ndarray:
    # Mfull[vi, vo] = 1 where (vi - vo) mod 256 in {3..10, 246..253}
    # (0/1 exactly representable in bf16; the alpha/16 scale is applied
    # in fp32 on the PSUM->SBUF eviction)
    import ml_dtypes

    d = np.arange(128)[:, None] - np.arange(128)[None, :]

    def f(dm):
        dm = np.mod(dm, 256)
        return ((dm >= 3) & (dm <= 10)) | ((dm >= 246) & (dm <= 253))

    w_diag = f(d).astype(np.float32)
    w_cross = f(d + 128).astype(np.float32)
    return np.ascontiguousarray(
        np.stack([w_diag, w_cross]).astype(ml_dtypes.bfloat16)
    )


def build_kernel():
    nc = bacc.Bacc(
        "TRN2",
        target_bir_lowering=False,
        debug=False,
        enable_asserts=False,
        num_devices=NCORES,
    )
    data = nc.dram_tensor("data", [ROWS, R], F32, kind="ExternalInput").ap()
    caw = nc.dram_tensor("caw", [2, 128, 128], BF16, kind="ExternalInput").ap()
    out = nc.dram_tensor("out", [ROWS, R], F32, kind="ExternalOutput").ap()

    COPY = mybir.ActivationFunctionType.Copy

    with TileContext(nc) as tc, ExitStack() as ctx:
        cpool = ctx.enter_context(tc.tile_pool(name="const", bufs=1))
        iopool = ctx.enter_context(tc.tile_pool(name="io", bufs=4))
        wpool = ctx.enter_context(tc.tile_pool(name="work", bufs=1))
        ospool = ctx.enter_context(tc.tile_pool(name="os", bufs=1))
        ppool = ctx.enter_context(tc.tile_pool(name="psum", bufs=4, space="PSUM"))
        opool = ctx.enter_context(tc.tile_pool(name="outb", bufs=2))

        # constants: min-gate plane for the first scan (+BIG, -BIG at pads)
        gate = cpool.tile([128, W1H], BF16)
        nc.vector.memset(gate[:], 1e30)
        gate3 = gate.rearrange("p (b c) -> p b c", c=PGW)
        nc.vector.memset(gate3[:, :, 0:PADS], NEG)
        # CA circulant weight blocks [vi, vo] (0/1 in bf16); loaded after
        # tile 0's data DMA so they don't delay the first scan chain
        w_sb = cpool.tile([128, 256], BF16)

        os_tiles = {}
        for t in range(NT):
            rows = slice(128 * t, 128 * t + 128)
            # ---- load (circular range halo) ----
            xc = iopool.tile([128, XC], F32, tag="xc")
            eng_a = nc.gpsimd if t == 0 else nc.sync
            eng_b = nc.scalar if t == 0 else nc.sync
            eng_a.dma_start(out=xc[:, 0:HALO], in_=data[rows, R - HALO : R])
            nc.sync.dma_start(out=xc[:, HALO : HALO + R], in_=data[rows, :])
            eng_b.dma_start(out=xc[:, HALO + R : XC], in_=data[rows, 0:HALO])
            xc3 = xc.rearrange("p (b c) -> p b c", c=16)
            if t == 0:
                nc.sync.dma_start(out=w_sb[:, 0:128], in_=caw[0])
                nc.sync.dma_start(out=w_sb[:, 128:256], in_=caw[1])

            # ---- padded page layout (fwd only; suffixes scan backward) ----
            xpr = wpool.tile([128, W1H], BF16, tag=f"xpr{t % 2}", name="xpr")
            xp3 = xpr.rearrange("p (b c) -> p b c", c=PGW)
            nc.gpsimd.memset(xp3[:, :, 0:PADS], NEG)
            if t == 0:
                nc.vector.tensor_copy(out=xp3[:, :, PADS:PGW], in_=xc3[:])
            else:
                nc.scalar.activation(out=xp3[:, :, PADS:PGW], in_=xc3[:], func=COPY)

            # ---- per-block prefix/suffix sorted top-8 scan chains ----
            # m_k[c] = min(max(x[c], state), m_{k-1}[c -/+ 1]); the clean p0
            # pad column in m_{k-1} resets the state at each block boundary.
            mfbuf = wpool.tile([128, 8 * W1H], BF16, tag="mfbuf")
            mrbuf = wpool.tile([128, 8 * W1H], BF16, tag="mrbuf")
            MSTRIDE = W1H
            mf = [mfbuf[:, k * W1H : (k + 1) * W1H] for k in range(8)]
            mr = [mrbuf[:, k * W1H : (k + 1) * W1H] for k in range(8)]

            def rev(ap_t, start_col, n):
                return AP(ap_t.tensor, ap_t.offset + start_col,
                          [list(ap_t.ap[0]), [-1, n]])

            # slot-1 scans (segmented running max via min-gate)
            nc.vector.tensor_tensor_scan(
                out=mf[0][:], data0=gate[:], data1=xpr[:],
                initial=NEG, op0=MIN, op1=MAX,
            )
            nc.vector.tensor_tensor_scan(
                out=rev(mr[0], W1H - 1, W1H), data0=rev(gate, W1H - 1, W1H),
                data1=rev(xpr, W1H - 1, W1H),
                initial=NEG, op0=MIN, op1=MAX,
            )
            for k in range(1, 8):
                # forward (prefix) chain
                nc.vector.tensor_tensor_scan(
                    out=mf[k][:, 1:W1H], data0=xpr[:, 1:W1H],
                    data1=mf[k - 1][:, 0 : W1H - 1],
                    initial=NEG, op0=MAX, op1=MIN,
                )
                mfp = mf[k].rearrange("p (b c) -> p b c", c=PGW)
                nc.gpsimd.memset(mfp[:, :, 0:1], NEG)
                # backward (suffix) chain; last block's tail is never consumed
                nc.gpsimd.memset(mr[k][:, W1H - 1 : W1H], NEG)
                nc.vector.tensor_tensor_scan(
                    out=rev(mr[k], W1H - 2, W1H - 1),
                    data0=rev(xpr, W1H - 2, W1H - 1),
                    data1=rev(mr[k - 1], W1H - 1, W1H - 1),
                    initial=NEG, op0=MAX, op1=MIN,
                )
                if k < 7:
                    # mr[7]'s pads are never read (no further chain; the
                    # valley's suffix view only touches data columns)
                    mrp = mr[k].rearrange("p (b c) -> p b c", c=PGW)
                    nc.gpsimd.memset(mrp[:, :, 0:1], NEG)

            # ---- valley planes: top-8 multiset of each 16-window ----
            # col c (within plane i) = window starting at xpad col c (s=c-32)
            # one batched op over all 8 slots:
            #   va[i] = max( mr[i] suffix view, mf[7-i] prefix view )
            # suffix of block b len 16-j -> mr col 17b + 1 + j
            # prefix of block b+1 len j  -> mf col 17b + 17 + j (j=0 -> p0 pad)
            # mf/mr tiles are allocated back-to-back per direction so a plane
            # stride can batch across slots. va/vb are 8-plane buffers.
            va = wpool.tile([128, 8 * VW], BF16, tag="va")

            vb = wpool.tile([128, 8 * VW], BF16, tag="vb")
            suf = AP(mr[0].tensor, mr[0].offset + 1,
                     [list(mr[0].ap[0]), [MSTRIDE, 8], [PGW, VB], [1, 16]])
            pre = AP(mf[7].tensor, mf[7].offset + PGW,
                     [list(mf[7].ap[0]), [-MSTRIDE, 8], [PGW, VB], [1, 16]])
            dst = va.rearrange("p (s b c) -> p s b c", b=VB, c=16)
            nc.vector.tensor_tensor(out=dst[:], in0=suf, in1=pre, op=MAX)

            def planes(buf, plist, width=VW, off=0):
                # uniform-stride plane list, or 2x2 block structure (e.g. 0,1,4,5)
                base = plist[0]
                if len(plist) == 4 and plist[1] - plist[0] != plist[3] - plist[2]:
                    raise AssertionError(plist)
                if len(plist) == 4 and plist[2] - plist[0] != plist[1] - plist[0] * 0 + (
                    plist[1] - plist[0]
                ) * 2:
                    inner = plist[1] - plist[0]
                    outer = plist[2] - plist[0]
                    return AP(buf.tensor, buf.offset + base * VW + off,
                              [list(buf.ap[0]), [outer * VW, 2], [inner * VW, 2], [1, width]])
                step = plist[1] - plist[0] if len(plist) > 1 else 1
                return AP(buf.tensor, buf.offset + base * VW + off,
                          [list(buf.ap[0]), [step * VW, len(plist)], [1, width]])

            # ---- bitonic resort of the valley (descending), batched ----
            for srcs, dsts, lo, hi in (
                (va, vb, (0, 1, 2, 3), (4, 5, 6, 7)),
                (vb, va, (0, 1, 4, 5), (2, 3, 6, 7)),
                (va, vb, (0, 2, 4, 6), (1, 3, 5, 7)),
            ):
                nc.vector.tensor_tensor(
                    out=planes(dsts, lo), in0=planes(srcs, lo), in1=planes(srcs, hi), op=MAX)
                nc.vector.tensor_tensor(
                    out=planes(dsts, hi), in0=planes(srcs, lo), in1=planes(srcs, hi), op=MIN)

            # ---- final OS merge of the two windows ----
            # F_i = max(T_i[col r+12], T_{7-i}[col r+37]); then min-tree
            if t % 2 == 0:
                ca_ps = {}
            else:
                ca_outp = {h: opool.tile([128, R], F32, tag=f"outp{h}", name=f"outp{h}")
                           for h in (0, 1)}
            os_t = ospool.tile([128, R], BF16, tag=f"os{t}", name=f"os{t}")
            for ch in (0, 1):
                co = 512 * ch
                rev8 = AP(vb.tensor, vb.offset + 7 * VW + 37 + co,
                          [list(vb.ap[0]), [-VW, 8], [1, 512]])
                fw8 = AP(vb.tensor, vb.offset + 12 + co,
                         [list(vb.ap[0]), [VW, 8], [1, 512]])
                fdst = AP(va.tensor, va.offset + co, [list(va.ap[0]), [VW, 8], [1, 512]])
                nc.vector.tensor_tensor(out=fdst, in0=fw8, in1=rev8, op=MAX)
                nc.vector.tensor_tensor(
                    out=planes(va, (0, 1, 2, 3), width=512, off=co),
                    in0=planes(va, (0, 1, 2, 3), width=512, off=co),
                    in1=planes(va, (4, 5, 6, 7), width=512, off=co), op=MIN)
                nc.vector.tensor_tensor(
                    out=planes(va, (0, 1), width=512, off=co),
                    in0=planes(va, (0, 1), width=512, off=co),
                    in1=planes(va, (2, 3), width=512, off=co), op=MIN)
                nc.vector.tensor_tensor(
                    out=os_t[:, co : co + 512], in0=planes(va, (0,), width=512, off=co),
                    in1=planes(va, (1,), width=512, off=co), op=MIN)
                # CA accumulation for this col-half right away (PE overlaps DVE)
                cols = slice(co, co + 512)
                for half in (0, 1):
                    if t % 2 == 0:
                        w_first = w_sb[:, 0:128] if half == 0 else w_sb[:, 128:256]
                        ps = ppool.tile([128, 512], F32, tag="ps", name=f"ps{half}{ch}")
                        nc.tensor.matmul(
                            out=ps[:], lhsT=w_first, rhs=os_t[:, cols],
                            start=True, stop=False,
                        )
                        ca_ps[(half, ch)] = ps
                    else:
                        w_second = w_sb[:, 128:256] if half == 0 else w_sb[:, 0:128]
                        ps = ca_ps[(half, ch)]
                        nc.tensor.matmul(
                            out=ps[:], lhsT=w_second, rhs=os_t[:, cols],
                            start=False, stop=True,
                        )
                        outp = ca_outp[half]
                        nc.scalar.activation(
                            out=outp[:, cols], in_=ps[:], func=COPY,
                            scale=float(OS_ALPHA / (2 * T[0])),
                        )
                        orows = slice(128 * (t - 1 + half), 128 * (t + half))
                        nc.sync.dma_start(out=out[orows, cols], in_=outp[:, cols])
            os_tiles[t] = os_t

    nc.compile()
    return nc


_NC_CACHE = None


def _get_nc():
    global _NC_CACHE
    if _NC_CACHE is None:
        _NC_CACHE = build_kernel()
    return _NC_CACHE


def run(data: np.ndarray, trace: bool = False, trace_kwargs=None):
    data = np.ascontiguousarray(np.asarray(data, dtype=np.float32))
    assert data.shape == (B, V, R), data.shape
    nc = _get_nc()
    caw = _ca_weights()
    in_maps = [
        {"data": np.ascontiguousarray(data[BPC * c : BPC * (c + 1)].reshape(ROWS, R)),
         "caw": caw}
        for c in range(NCORES)
    ]
    try:
        res = bass_utils.run_bass_kernel_spmd(
            nc, in_maps, core_ids=list(range(NCORES)),
            trace=trace, **(trace_kwargs or {}),
        )
    except ModuleNotFoundError:
        # no NTFF hook in this environment -- run without tracing
        res = bass_utils.run_bass_kernel_spmd(
            nc, in_maps, core_ids=list(range(NCORES)), trace=False,
        )
    outs = [res.results[c]["out"].reshape(BPC, V, R) for c in range(NCORES)]
    return np.concatenate(outs, axis=0), res


def kernel(data: np.ndarray) -> np.ndarray:
    out, _ = run(data)
    return out
